# revision 35
# baseline (speedup 1.0000x reference)
"""Trainium2 Bass kernel for the deterministic legality module.

Computes, for each board b, filter f and top-left placement (i,j):
    legal[b,f,i,j] = 1.0 iff every occupied cell of filter f, placed at
    (i,j), lands in-bounds on a free cell of board b (and f is non-empty).

Reformulated as one matmul per output tile:
    out[b, f*81+ij] = relu( sum_k boardX[b,k] * M[k, f*81+ij] )
where rows 0..80 of M hold filter f placed at ij (zero out of bounds) and
rows 81,82 hold the two integer halves of thr[f] = 1-area (or -1 for empty
filters); boardX appends two ones-columns to the board.  corr <= area
always, so relu(corr + thr) is exactly the 0/1 legality.

Both M and the transposed board are built on the HOST in numpy and
uploaded in fp8e4 (entries are 0/1 or integers in [-12,0] -- exact in
e4m3), padded to 128 partitions: the DMA engines have fixed partition
affinity, so 128-partition transfers fan out across all 16 engines while
narrow ones serialize.  The device is then a single pipeline:
  matmul (fp8, PSUM f32) -> relu+fp8 downcast (DVE/ACT, 5:6 split)
  -> HBM store (fp8, upcast to f32 on host).
All matmuls contract over the full 128 partitions (pad rows are zero, and
matmul time is N-bound) because the HAM clock gate keys on PE array row
occupancy: partial-K matmuls re-throttle the PE to 1.2 GHz even at 100%
busy, while full-K work holds it at 2.4 GHz.

Sharding: pure data parallelism, batch 4096 -> 512 per core on 8 cores.
"""

import numpy as np
import ml_dtypes

N_CORES = 8
BATCH = 4096
BPC = BATCH // N_CORES  # 512 boards per core
NPOS = 81               # 9x9 board cells / placements
NF = 264                # filters
NCOL = NF * NPOS        # 21384 output columns per board
K = NPOS + 2            # contraction: 81 board cells + 2 threshold rows
KPAD = 128              # uploads padded to 128 partitions for DMA fan-out

COL_TILE = 512          # one PSUM bank of f32
GRP = 1024              # 2 banks per PSUM ring slot / relu op
DMA_GRP = 4096          # output staging tile / store DMA (tail stays 904)
N_SLABS = 8             # M upload slabs after the 512-col prefetch slab
# The HAM clock gate keys on PE array occupancy: K=83 matmuls (65% of the
# 128 rows) read as "idle" and the clock re-throttles to 1.2 GHz even when
# the PE is 100% busy.  All matmuls therefore contract over the full 128
# partitions -- the pad rows are zero on both sides, and matmul time is
# N-bound, so K=128 is free, self-lifts the gate ~3.4us into the main
# loop, and holds it at 2.4 GHz (no separate warm-up needed).
# DVE:ACT relu split, retuned from measured op times (1209 vs 1083 ns)
DVE_SLOTS = (0, 2, 4, 6, 8, 10, 12, 14)
PERIOD = 17


def _build_m(filters: np.ndarray, areas: np.ndarray) -> np.ndarray:
    """M [128, 21384] fp8e4: placed-filter geometry + threshold rows + pad."""
    F = np.asarray(filters, dtype=np.float32).reshape(NF, 5, 5)
    M = np.zeros((KPAD, NF, NPOS), dtype=np.float32)
    for i in range(9):
        h = min(5, 9 - i)
        for j in range(9):
            w = min(5, 9 - j)
            blk = np.zeros((NF, 9, 9), dtype=np.float32)
            blk[:, i:i + h, j:j + w] = F[:, :h, :w]
            M[:NPOS, :, i * 9 + j] = blk.reshape(NF, NPOS).T
    ar = np.asarray(areas, dtype=np.float32).reshape(NF)
    thr = np.where(ar > 0.5, 1.0 - ar, -1.0)
    lo = np.floor(thr / 2.0)
    M[NPOS, :, :] = lo[:, None]          # floor(thr/2)   in [-12, 0]
    M[NPOS + 1, :, :] = (thr - lo)[:, None]  # ceil(thr/2) in [-12, 0]
    return M.reshape(KPAD, NCOL).astype(ml_dtypes.float8_e4m3)


def _build_boardt(board_free: np.ndarray) -> np.ndarray:
    """boardT [cores, 128, 512] fp8e4: transposed boards + ones rows + pad."""
    b = np.asarray(board_free, dtype=np.float32).reshape(N_CORES, BPC, NPOS)
    bt = np.zeros((N_CORES, KPAD, BPC), dtype=np.float32)
    bt[:, :NPOS, :] = b.transpose(0, 2, 1)
    bt[:, NPOS:K, :] = 1.0
    return bt.astype(ml_dtypes.float8_e4m3)


def _build_module():
    import concourse.bass as bass
    import concourse.mybir as mybir
    import concourse.tile as tile

    f32 = mybir.dt.float32
    fp8 = mybir.dt.float8e4
    Relu = mybir.ActivationFunctionType.Relu

    nc = bass.Bass("TRN2", target_bir_lowering=False, debug=False,
                   num_devices=N_CORES)

    boardt_d = nc.dram_tensor("boardt", [KPAD, BPC], fp8, kind="ExternalInput")
    m_d = nc.dram_tensor("mmat", [KPAD, NCOL], fp8, kind="ExternalInput")
    out_d = nc.dram_tensor("out", [BPC, NCOL], fp8, kind="ExternalOutput")

    with tile.TileContext(nc) as tc:
        with tc.tile_pool(name="const", bufs=1) as cpool:
            boardT = cpool.tile([KPAD, BPC], fp8)
            msb = cpool.tile([KPAD, NCOL], fp8)

            # M slabs on the SP hwdge ring (slab 0 configured first so the
            # first matmuls start as early as possible), boardT on the ACT
            # ring so both uploads run in parallel; output stores follow on
            # the SP ring, FIFO behind the slabs they never contend with.
            bounds = [0, 512]  # tiny prefetch slab: earliest first matmul
            step = (NCOL - 512) // N_SLABS
            while len(bounds) <= N_SLABS:
                bounds.append(bounds[-1] + step)
            bounds[-1] = NCOL
            for s0, s1 in zip(bounds[:-1], bounds[1:]):
                nc.sync.dma_start(msb[:, s0:s1], m_d[:, s0:s1])
            nc.scalar.dma_start(boardT[:], boardt_d[:])

            # ---- pipeline: PSUM ring (4 slots) + staging ---------------
            with (
                tc.tile_pool(name="wprep", bufs=1) as wprep,
                tc.tile_pool(name="psM", bufs=4, space="PSUM") as psM,
                tc.tile_pool(name="ostage", bufs=6) as ostage,
            ):
                # Preload the ACT activation table so the first real relu
                # doesn't pay the ~1.3us table load; source from a memset
                # scratch so it can run during the input upload.
                wrd = wprep.tile([32, 1], f32, tag="wrd")
                wz = wprep.tile([32, 1], f32, tag="wz")
                nc.vector.memset(wz[:], 0.0)
                nc.scalar.activation(wrd[0:1, 0:1], wz[0:1, 0:1], Relu)

                grp = 0
                nkb = BPC // 128
                for kb in range(nkb):
                    lhsT = boardT[:, kb * 128:(kb + 1) * 128]
                    for g0 in range(0, NCOL, DMA_GRP):
                        dw = min(DMA_GRP, NCOL - g0)
                        last = kb == nkb - 1 and g0 + dw >= NCOL
                        ot = ostage.tile([128, DMA_GRP], fp8, tag="ot")
                        for h0 in range(0, dw, GRP):
                            hw = min(GRP, dw - h0)
                            pt = psM.tile([128, GRP], f32, tag="mm")
                            for q in range(0, hw, COL_TILE):
                                w = min(COL_TILE, hw - q)
                                c = g0 + h0 + q
                                nc.tensor.matmul(pt[:, q:q + w], lhsT,
                                                 msb[:, c:c + w],
                                                 start=True, stop=True)
                            if last and h0 + hw >= dw:
                                # final group: drain on both engines so the
                                # closing store starts as early as possible
                                hh = hw // 2
                                nc.vector.tensor_scalar_max(
                                    ot[:, h0:h0 + hh], pt[:, :hh], 0.0)
                                nc.scalar.activation(ot[:, h0 + hh:h0 + hw],
                                                     pt[:, hh:hw], Relu)
                            elif grp % PERIOD in DVE_SLOTS:
                                nc.vector.tensor_scalar_max(
                                    ot[:, h0:h0 + hw], pt[:, :hw], 0.0)
                            else:
                                nc.scalar.activation(ot[:, h0:h0 + hw],
                                                     pt[:, :hw], Relu)
                            grp += 1
                        # the final store goes on the otherwise-idle ACT
                        # ring so it never queues behind the prior 4 KB
                        # transfer still draining on the SP ring
                        eng = nc.scalar if last else nc.sync
                        eng.dma_start(
                            out_d[kb * 128:(kb + 1) * 128, g0:g0 + dw],
                            ot[:, :dw])
    return nc


def _legalize_multiwait(nc):
    """Split multi-wait instructions for this walrus build.

    The TPB instruction encodings carry exactly one semaphore wait, and
    the walrus codegen here refuses instructions with more ("Too many
    sync wait commands").  Hoist all but one wait onto EventSemaphore
    carrier instructions placed immediately before, on the same engine —
    the sequencer blocks on each carrier first, which is semantically
    identical.
    """
    import concourse.mybir as mybir

    for func in nc.m.functions:
        for blk in func.blocks:
            out = []
            changed = False
            for inst in blk.instructions:
                si = inst.sync_info
                waits = list(si.on_wait) if si is not None and si.on_wait else []
                if len(waits) > 1:
                    for j, w in enumerate(waits[:-1]):
                        carrier = mybir.InstEventSemaphore(
                            name=f"{inst.name}-xw{j}",
                            engine=inst.engine,
                            ins=[], outs=[],
                            sync_info=mybir.SyncInfo(on_wait=[w],
                                                     on_update=[]),
                        )
                        nc.register_instruction(carrier)
                        out.append(carrier)
                    inst.sync_info = mybir.SyncInfo(
                        on_wait=[waits[-1]],
                        on_update=list(si.on_update) if si.on_update else [])
                    changed = True
                out.append(inst)
            if changed:
                blk.instructions = out


_MODULE = None


def _get_module():
    global _MODULE
    if _MODULE is None:
        _MODULE = _build_module()
        _legalize_multiwait(_MODULE)
    return _MODULE


def run(board_free, filters, areas, trace=False, **spmd_kwargs):
    from concourse.bass_utils import run_bass_kernel_spmd

    boardt = _build_boardt(board_free)
    mmat = _build_m(filters, areas)

    in_maps = [
        {"boardt": boardt[c], "mmat": mmat}
        for c in range(N_CORES)
    ]
    nc = _get_module()
    res = run_bass_kernel_spmd(nc, in_maps, core_ids=list(range(N_CORES)),
                               trace=trace, **spmd_kwargs)
    out = np.concatenate(
        [np.asarray(r["out"]).astype(np.float32) for r in res.results], axis=0)
    out = out.reshape(BATCH, NF, 9, 9)
    return out, res


def kernel(board_free, filters, areas):
    out, _ = run(board_free, filters, areas)
    return out


# revision 36
# speedup vs baseline: 1.0021x; 1.0021x over previous
"""Trainium2 Bass kernel for the deterministic legality module.

Computes, for each board b, filter f and top-left placement (i,j):
    legal[b,f,i,j] = 1.0 iff every occupied cell of filter f, placed at
    (i,j), lands in-bounds on a free cell of board b (and f is non-empty).

Reformulated as one matmul per output tile:
    out[b, f*81+ij] = relu( sum_k boardX[b,k] * M[k, f*81+ij] )
where rows 0..80 of M hold filter f placed at ij (zero out of bounds) and
rows 81,82 hold the two integer halves of thr[f] = 1-area (or -1 for empty
filters); boardX appends two ones-columns to the board.  corr <= area
always, so relu(corr + thr) is exactly the 0/1 legality.

Both M and the transposed board are built on the HOST in numpy and
uploaded in fp8e4 (entries are 0/1 or integers in [-12,0] -- exact in
e4m3), padded to 128 partitions: the DMA engines have fixed partition
affinity, so 128-partition transfers fan out across all 16 engines while
narrow ones serialize.  The device is then a single pipeline:
  matmul (fp8, PSUM f32) -> relu+fp8 downcast (DVE/ACT, 5:6 split)
  -> HBM store (fp8, upcast to f32 on host).
All matmuls contract over the full 128 partitions (pad rows are zero, and
matmul time is N-bound) because the HAM clock gate keys on PE array row
occupancy: partial-K matmuls re-throttle the PE to 1.2 GHz even at 100%
busy, while full-K work holds it at 2.4 GHz.

Sharding: pure data parallelism, batch 4096 -> 512 per core on 8 cores.
"""

import numpy as np
import ml_dtypes

N_CORES = 8
BATCH = 4096
BPC = BATCH // N_CORES  # 512 boards per core
NPOS = 81               # 9x9 board cells / placements
NF = 264                # filters
NCOL = NF * NPOS        # 21384 output columns per board
K = NPOS + 2            # contraction: 81 board cells + 2 threshold rows
KPAD = 128              # uploads padded to 128 partitions for DMA fan-out

COL_TILE = 512          # one PSUM bank of f32
GRP = 1024              # 2 banks per PSUM ring slot / relu op
DMA_GRP = 4096          # output staging tile / store DMA (tail stays 904)
N_SLABS = 8             # M upload slabs after the 512-col prefetch slab
# The HAM clock gate keys on PE array occupancy: K=83 matmuls (65% of the
# 128 rows) read as "idle" and the clock re-throttles to 1.2 GHz even when
# the PE is 100% busy.  All matmuls therefore contract over the full 128
# partitions -- the pad rows are zero on both sides, and matmul time is
# N-bound, so K=128 is free, self-lifts the gate ~3.4us into the main
# loop, and holds it at 2.4 GHz (no separate warm-up needed).
# DVE:ACT relu split, retuned from measured op times (1209 vs 1083 ns)
DVE_SLOTS = (0, 2, 4, 6, 8, 10, 12, 14)
PERIOD = 17


def _build_m(filters: np.ndarray, areas: np.ndarray) -> np.ndarray:
    """M [128, 21384] fp8e4: placed-filter geometry + threshold rows + pad."""
    F = np.asarray(filters, dtype=np.float32).reshape(NF, 5, 5)
    M = np.zeros((KPAD, NF, NPOS), dtype=np.float32)
    for i in range(9):
        h = min(5, 9 - i)
        for j in range(9):
            w = min(5, 9 - j)
            blk = np.zeros((NF, 9, 9), dtype=np.float32)
            blk[:, i:i + h, j:j + w] = F[:, :h, :w]
            M[:NPOS, :, i * 9 + j] = blk.reshape(NF, NPOS).T
    ar = np.asarray(areas, dtype=np.float32).reshape(NF)
    thr = np.where(ar > 0.5, 1.0 - ar, -1.0)
    lo = np.floor(thr / 2.0)
    M[NPOS, :, :] = lo[:, None]          # floor(thr/2)   in [-12, 0]
    M[NPOS + 1, :, :] = (thr - lo)[:, None]  # ceil(thr/2) in [-12, 0]
    return M.reshape(KPAD, NCOL).astype(ml_dtypes.float8_e4m3)


def _build_boardt(board_free: np.ndarray) -> np.ndarray:
    """boardT [cores, 128, 512] fp8e4: transposed boards + ones rows + pad."""
    b = np.asarray(board_free, dtype=np.float32).reshape(N_CORES, BPC, NPOS)
    bt = np.zeros((N_CORES, KPAD, BPC), dtype=np.float32)
    bt[:, :NPOS, :] = b.transpose(0, 2, 1)
    bt[:, NPOS:K, :] = 1.0
    return bt.astype(ml_dtypes.float8_e4m3)


def _build_module():
    import concourse.bass as bass
    import concourse.mybir as mybir
    import concourse.tile as tile

    f32 = mybir.dt.float32
    fp8 = mybir.dt.float8e4
    Relu = mybir.ActivationFunctionType.Relu

    nc = bass.Bass("TRN2", target_bir_lowering=False, debug=False,
                   num_devices=N_CORES)

    boardt_d = nc.dram_tensor("boardt", [KPAD, BPC], fp8, kind="ExternalInput")
    m_d = nc.dram_tensor("mmat", [KPAD, NCOL], fp8, kind="ExternalInput")
    out_d = nc.dram_tensor("out", [BPC, NCOL], fp8, kind="ExternalOutput")

    with tile.TileContext(nc) as tc:
        with tc.tile_pool(name="const", bufs=1) as cpool:
            boardT = cpool.tile([KPAD, BPC], fp8)
            msb = cpool.tile([KPAD, NCOL], fp8)

            # M slabs on the SP hwdge ring (slab 0 configured first so the
            # first matmuls start as early as possible), boardT on the ACT
            # ring so both uploads run in parallel; output stores follow on
            # the SP ring, FIFO behind the slabs they never contend with.
            bounds = [0, 512]  # tiny prefetch slab: earliest first matmul
            step = (NCOL - 512) // N_SLABS
            while len(bounds) <= N_SLABS:
                bounds.append(bounds[-1] + step)
            bounds[-1] = NCOL
            for s0, s1 in zip(bounds[:-1], bounds[1:]):
                nc.sync.dma_start(msb[:, s0:s1], m_d[:, s0:s1])
            nc.scalar.dma_start(boardT[:], boardt_d[:])

            # ---- pipeline: PSUM ring (4 slots) + staging ---------------
            with (
                tc.tile_pool(name="wprep", bufs=1) as wprep,
                tc.tile_pool(name="psM", bufs=4, space="PSUM") as psM,
                tc.tile_pool(name="ostage", bufs=6) as ostage,
            ):
                # Preload the ACT activation table so the first real relu
                # doesn't pay the ~1.3us table load; source from a memset
                # scratch so it can run during the input upload.
                wrd = wprep.tile([32, 1], f32, tag="wrd")
                wz = wprep.tile([32, 1], f32, tag="wz")
                nc.vector.memset(wz[:], 0.0)
                nc.scalar.activation(wrd[0:1, 0:1], wz[0:1, 0:1], Relu)

                grp = 0
                nkb = BPC // 128
                for kb in range(nkb):
                    lhsT = boardT[:, kb * 128:(kb + 1) * 128]
                    # smaller stores on the final chunk: they release more
                    # often, so the DMA stream finishes with the drains
                    # instead of 3-4us after them
                    dgrp = DMA_GRP if kb < nkb - 1 else DMA_GRP // 2
                    for g0 in range(0, NCOL, dgrp):
                        dw = min(dgrp, NCOL - g0)
                        last = kb == nkb - 1 and g0 + dw >= NCOL
                        ot = ostage.tile([128, DMA_GRP], fp8, tag="ot")
                        for h0 in range(0, dw, GRP):
                            hw = min(GRP, dw - h0)
                            pt = psM.tile([128, GRP], f32, tag="mm")
                            for q in range(0, hw, COL_TILE):
                                w = min(COL_TILE, hw - q)
                                c = g0 + h0 + q
                                nc.tensor.matmul(pt[:, q:q + w], lhsT,
                                                 msb[:, c:c + w],
                                                 start=True, stop=True)
                            if last and h0 + hw >= dw:
                                # final group: drain on both engines so the
                                # closing store starts as early as possible
                                hh = hw // 2
                                nc.vector.tensor_scalar_max(
                                    ot[:, h0:h0 + hh], pt[:, :hh], 0.0)
                                nc.scalar.activation(ot[:, h0 + hh:h0 + hw],
                                                     pt[:, hh:hw], Relu)
                            elif grp % PERIOD in DVE_SLOTS:
                                nc.vector.tensor_scalar_max(
                                    ot[:, h0:h0 + hw], pt[:, :hw], 0.0)
                            else:
                                nc.scalar.activation(ot[:, h0:h0 + hw],
                                                     pt[:, :hw], Relu)
                            grp += 1
                        # the final store goes on the otherwise-idle ACT
                        # ring so it never queues behind the prior 4 KB
                        # transfer still draining on the SP ring
                        eng = nc.scalar if last else nc.sync
                        eng.dma_start(
                            out_d[kb * 128:(kb + 1) * 128, g0:g0 + dw],
                            ot[:, :dw])
    return nc


def _legalize_multiwait(nc):
    """Split multi-wait instructions for this walrus build.

    The TPB instruction encodings carry exactly one semaphore wait, and
    the walrus codegen here refuses instructions with more ("Too many
    sync wait commands").  Hoist all but one wait onto EventSemaphore
    carrier instructions placed immediately before, on the same engine —
    the sequencer blocks on each carrier first, which is semantically
    identical.
    """
    import concourse.mybir as mybir

    for func in nc.m.functions:
        for blk in func.blocks:
            out = []
            changed = False
            for inst in blk.instructions:
                si = inst.sync_info
                waits = list(si.on_wait) if si is not None and si.on_wait else []
                if len(waits) > 1:
                    for j, w in enumerate(waits[:-1]):
                        carrier = mybir.InstEventSemaphore(
                            name=f"{inst.name}-xw{j}",
                            engine=inst.engine,
                            ins=[], outs=[],
                            sync_info=mybir.SyncInfo(on_wait=[w],
                                                     on_update=[]),
                        )
                        nc.register_instruction(carrier)
                        out.append(carrier)
                    inst.sync_info = mybir.SyncInfo(
                        on_wait=[waits[-1]],
                        on_update=list(si.on_update) if si.on_update else [])
                    changed = True
                out.append(inst)
            if changed:
                blk.instructions = out


_MODULE = None


def _get_module():
    global _MODULE
    if _MODULE is None:
        _MODULE = _build_module()
        _legalize_multiwait(_MODULE)
    return _MODULE


def run(board_free, filters, areas, trace=False, **spmd_kwargs):
    from concourse.bass_utils import run_bass_kernel_spmd

    boardt = _build_boardt(board_free)
    mmat = _build_m(filters, areas)

    in_maps = [
        {"boardt": boardt[c], "mmat": mmat}
        for c in range(N_CORES)
    ]
    nc = _get_module()
    res = run_bass_kernel_spmd(nc, in_maps, core_ids=list(range(N_CORES)),
                               trace=trace, **spmd_kwargs)
    out = np.concatenate(
        [np.asarray(r["out"]).astype(np.float32) for r in res.results], axis=0)
    out = out.reshape(BATCH, NF, 9, 9)
    return out, res


def kernel(board_free, filters, areas):
    out, _ = run(board_free, filters, areas)
    return out
